# revision 1
# baseline (speedup 1.0000x reference)
"""TRN2 Bass kernel for nn_AttentionStoreProcessor (dense transformer attention).

Full (unsharded) inputs in, full output out. Internally:
  - CAPE rotation + softmax scale folded into Wq/Wk on host (exact linear algebra,
    per-frame 4x4 block-diagonal right-multiply).
  - Heads padded 20 -> 24 and tensor-parallel sharded 3 heads/core across 8 cores
    (zero weights for pad heads; their output contribution is exactly zero).
  - Per core: hs^T via PE transposes; fused QKV projections (float32r ~= tf32
    precision at full PE rate); scores^T per (head, kt-tile); max-free softmax
    (scores are O(10), exp is safe in fp32) with sums obtained via a ones-column
    appended to V in the PV matmul; per-query normalization via a K=1 broadcast
    matmul; output projection from outT, overlapped per query-half; residual,
    bias and the cross-core partial-sum reduction happen on host.
"""
import numpy as np
from contextlib import ExitStack

import concourse.bacc as bacc
import concourse.mybir as mybir
import concourse.tile as tile
from concourse.bass_utils import run_bass_kernel_spmd

F32 = mybir.dt.float32
F32R = mybir.dt.float32r
AF = mybir.ActivationFunctionType

HEADS = 20
PAD_HEADS = 24
HPC = 3  # heads per core
N_CORES = 8
S = 2048  # tokens
D = 1280  # channels
HD = 64  # head dim
L = 1024  # tokens per frame
KT = D // 128  # 10 contraction tiles for projections
TOKT = S // 128  # 16 token tiles

# wpack free-dim layout (per partition):
#   [0:7680)      six 1280-wide wg blocks, order (t0g0,t0g1,t0g2,t1g0,t1g1,t1g2)
#   [7680:10240)  wv, KT tiles of 256 cols ([v_h0|v_h1|v_h2|zeros(64)])
#   [10240:10368) identity 128x128
#   [10368:10432) ones 128x64
WV_OFF = 7680
ID_OFF = 10240
ONES_OFF = 10368
WPACK_W = 10432

_CACHED_NC = None


def _build_nc():
    nc = bacc.Bacc("TRN2", debug=False, num_devices=N_CORES)

    hs = nc.dram_tensor("hs", [S, D], F32R, kind="ExternalInput").ap()
    wpack = nc.dram_tensor("wpack", [128, WPACK_W], F32R, kind="ExternalInput").ap()
    wopack = nc.dram_tensor("wopack", [128, 2560], F32R, kind="ExternalInput").ap()
    out = nc.dram_tensor("out", [S, D], F32, kind="ExternalOutput").ap()

    hs_r = hs.rearrange("(n p) d -> n p d", p=128)
    out_r = out.rearrange("(n p) d -> n p d", p=128)

    with (
        tile.TileContext(nc) as tc,
        ExitStack() as ctx,
        nc.allow_low_precision(reason="float32r (~tf32) used deliberately"),
    ):
        persist = ctx.enter_context(tc.tile_pool(name="persist", bufs=1))
        hsin_pool = tc.alloc_tile_pool(name="hsin", bufs=7)
        psT = tc.alloc_tile_pool(name="psT", bufs=8, space="PSUM")
        s1 = tc.alloc_tile_pool(name="s1", bufs=1)

        # identity + ones first (small DMA on the ACT ring so transposes can
        # start as soon as the first hs tile lands on the SP ring)
        io_sb = s1.tile([128, 192], F32R, tag="identones")
        nc.scalar.dma_start(io_sb[:], wpack[:, ID_OFF:WPACK_W])
        ident_sb = io_sb[:, 0:128]
        ones_sb = persist.tile([128, 64], F32R, tag="ones")
        nc.vector.tensor_copy(ones_sb[:], io_sb[:, 128:192])

        # hs tiles: SP ring, emitted before the big weight DMA
        hs_sb = []
        for n in range(TOKT):
            t_in = hsin_pool.tile([128, D], F32R, tag="hsin", name=f"hsin{n}")
            eng = nc.sync if n % 2 == 0 else nc.scalar
            eng.dma_start(t_in[:], hs_r[n])
            hs_sb.append(t_in)

        # projection weights (single big DMA, lands while transposes run)
        wp = s1.tile([128, ID_OFF], F32R, tag="wpack")
        nc.sync.dma_start(wp[:], wpack[:, 0:ID_OFF])
        wg_sb = [
            [wp[:, (t * 3 + g) * 1280 : (t * 3 + g + 1) * 1280] for g in range(3)]
            for t in range(2)
        ]
        wv_sb = wp[:, WV_OFF:ID_OFF]

        hsT = [s1.tile([128, S], F32R, tag=f"hsT{k}", name=f"hsT{k}") for k in range(KT)]
        QA = persist.tile([128, S], F32R, tag="QA")  # rows 0:64 qT_h0, 64:128 qT_h1
        KA = persist.tile([128, S], F32R, tag="KA")  # rows 0:64 kT_h0, 64:128 kT_h1
        QK2 = persist.tile([128, S], F32R, tag="QK2")  # rows 0:64 q2, 64:128 k2
        QB2 = persist.tile([128, S], F32R, tag="QB2")  # rows 64:128 <- q2 (shifted)
        v195 = persist.tile([128, TOKT, 195], F32R, tag="v195")

        # ones columns of v_ext (col 65h+64 = 1.0)
        for h in range(HPC):
            nc.vector.tensor_copy(v195[:, :, 65 * h + 64], ones_sb[:, 0:TOKT])

        # ---- stage T: PE-transpose hs into hsT (psum evacuation on ScalarE,
        # which is otherwise idle until the attention exps start) ----
        for grp in range(4):  # groups of 4 token tiles
            for k in range(KT):
                tp = psT.tile([128, 512], F32R, tag="ps512", name=f"tp{grp}_{k}")
                for j in range(4):
                    n = grp * 4 + j
                    nc.tensor.transpose(
                        tp[:, j * 128 : (j + 1) * 128],
                        hs_sb[n][:, k * 128 : (k + 1) * 128],
                        ident_sb,
                    )
                nc.scalar.copy(hsT[k][:, grp * 512 : (grp + 1) * 512], tp[:])

        # ---- stage P: projections ----
        # q/k groups: per 512-token chunk (4 chunks; chunk//2 selects CAPE frame t)
        for ch in range(4):
            t = ch // 2
            qs = slice(ch * 512, (ch + 1) * 512)
            for g, dest in enumerate((QA, KA, QK2)):
                pp = psT.tile([128, 512], F32, tag="ps512", name=f"pp{ch}_{g}")
                for k in range(KT):
                    nc.tensor.matmul(
                        pp[:],
                        wg_sb[t][g][:, k * 128 : (k + 1) * 128],
                        hsT[k][:, qs],
                        start=(k == 0),
                        stop=(k == KT - 1),
                    )
                nc.vector.tensor_copy(dest[:, qs], pp[:])
            # v for the 4 token tiles of this chunk (256-wide output keeps the
            # f32r matmul at 1 cyc/row; cols 192:256 are zero padding)
            for j in range(4):
                n = ch * 4 + j
                vp = psT.tile([128, 256], F32, tag="ps512", name=f"vp{n}")
                for k in range(KT):
                    nc.tensor.matmul(
                        vp[:],
                        hsT[k][:, n * 128 : (n + 1) * 128],
                        wv_sb[:, k * 256 : (k + 1) * 256],
                        start=(k == 0),
                        stop=(k == KT - 1),
                    )
                for h in range(HPC):
                    nc.vector.tensor_copy(
                        v195[:, n, 65 * h : 65 * h + 64],
                        vp[:, h * 64 : (h + 1) * 64],
                    )

        # shift q2 (QK2 rows 0:64) up to rows 64:128 so h2 scores run at base 64
        nc.sync.dma_start(QB2[64:128, :], QK2[0:64, :])

        # free stage-1 SBUF (hsT, projection weights, hs input staging)
        s1.release()
        psT.release()
        hsin_pool.release()

        # late-stage tensors (created after hsT frees up SBUF)
        persistB = ctx.enter_context(tc.tile_pool(name="persistB", bufs=1))
        u_pool = tc.alloc_tile_pool(name="u", bufs=6)
        rc_pool = tc.alloc_tile_pool(name="rc", bufs=3)
        osb_pool = tc.alloc_tile_pool(name="osb", bufs=6)
        outT01 = persistB.tile([128, S], F32R, tag="outT01")
        outT2 = persistB.tile([64, S], F32R, tag="outT2")
        oT1tmp = persistB.tile([64, S], F32R, tag="oT1tmp")
        wop = persistB.tile([128, 2560], F32R, tag="wop")
        nc.scalar.dma_start(wop[:], wopack[:])
        wo01_sb = wop[:, 0:1280]
        wo2_sb = wop[0:64, 1280:2560]

        sc_pool = tc.alloc_tile_pool(name="sc", bufs=2, space="PSUM")
        pv_pool = tc.alloc_tile_pool(name="pv", bufs=4, space="PSUM")

        def head_ops(h):
            # (kT source, rows, qT source, rows) — both at the same base
            if h == 0:
                return KA, slice(0, 64), QA, slice(0, 64)
            if h == 1:
                return KA, slice(64, 128), QA, slice(64, 128)
            return QK2, slice(64, 128), QB2, slice(64, 128)

        def score_pv(h, qh, kt, pv_tiles, name):
            ksrc, krows, qsrc, qrows = head_ops(h)
            sc = sc_pool.tile([128, 1024], F32, tag="sc", name=f"sc{name}")
            for half in range(2):
                nc.tensor.matmul(
                    sc[:, half * 512 : (half + 1) * 512],
                    ksrc[krows, kt * 128 : (kt + 1) * 128],
                    qsrc[
                        qrows,
                        qh * 1024 + half * 512 : qh * 1024 + (half + 1) * 512,
                    ],
                    start=True,
                    stop=True,
                )
            u = u_pool.tile([128, 1024], F32R, tag="u", name=f"u{name}")
            nc.scalar.activation(u[:], sc[:], AF.Exp)
            for sub in range(2):
                nc.tensor.matmul(
                    pv_tiles[sub],
                    v195[:, kt, 65 * h : 65 * h + 65],
                    u[:, sub * 512 : (sub + 1) * 512],
                    start=(kt == 0),
                    stop=(kt == TOKT - 1),
                )

        def normalize(h, qh, pv_tiles):
            for sub in range(2):
                pvt = pv_tiles[sub]
                qcol = slice(qh * 1024 + sub * 512, qh * 1024 + (sub + 1) * 512)
                nm = f"{h}_{qh}_{sub}"
                rc = rc_pool.tile([65, 512], F32R, tag="rc", name=f"rc{nm}")
                nc.vector.reciprocal(rc[64:65, :], pvt[64:65, :])
                bc = sc_pool.tile([64, 512], F32, tag="sc", name=f"bc{nm}")
                nc.tensor.matmul(
                    bc[:], ones_sb[64:65, :], rc[64:65, :], start=True, stop=True
                )
                bcs = rc_pool.tile([64, 512], F32, tag="bcs", name=f"bcs{nm}")
                nc.vector.tensor_copy(bcs[:], bc[:])
                if h == 0:
                    dest = outT01[0:64, qcol]
                elif h == 1:
                    dest = oT1tmp[:, qcol]
                else:
                    dest = outT2[:, qcol]
                nc.vector.tensor_mul(dest, pvt[0:64, :], bcs[:])

        def outproj(m):
            # output projection for token tiles 4m..4m+3; op psum borrows
            # pv-pool slots so the first half overlaps the second qh's attention
            ob = osb_pool.tile([128, D], F32, tag="osb", name=f"ob{m}")
            for j in range(4):
                n = m * 4 + j
                ts = slice(n * 128, (n + 1) * 128)
                if j > 0:
                    ob = osb_pool.tile([128, D], F32, tag="osb", name=f"ob{m}_{j}")
                for dc, (off, w) in enumerate(((0, 512), (512, 512), (1024, 256))):
                    op = pv_pool.tile([128, 512], F32, tag="pv", name=f"op{n}_{dc}")
                    nc.tensor.matmul(
                        op[:, 0:w],
                        outT01[:, ts],
                        wo01_sb[:, off : off + w],
                        start=True,
                        stop=False,
                    )
                    nc.tensor.matmul(
                        op[:, 0:w],
                        outT2[:, ts],
                        wo2_sb[:, off : off + w],
                        start=False,
                        stop=True,
                    )
                    if (n * 3 + dc) % 2 == 0:
                        nc.vector.tensor_copy(ob[:, off : off + w], op[:, 0:w])
                    else:
                        nc.scalar.copy(ob[:, off : off + w], op[:, 0:w])
                eng = nc.sync if n % 2 == 0 else nc.scalar
                eng.dma_start(out_r[n], ob[:])

        for qh in range(2):
            # heads 0,1 interleaved: their score matmuls occupy PE row groups
            # 0:64 / 64:128 and run concurrently
            pv01 = {
                h: [
                    pv_pool.tile([65, 512], F32, tag="pv", name=f"pv{qh}_{h}_{s_}")
                    for s_ in range(2)
                ]
                for h in range(2)
            }
            for kt in range(TOKT):
                for h in range(2):
                    score_pv(h, qh, kt, pv01[h], f"{qh}_{kt}_{h}")
            for h in range(2):
                normalize(h, qh, pv01[h])
            # h1's outT half into rows 64:128 of outT01 (partition-shift DMA)
            half = slice(qh * 1024, (qh + 1) * 1024)
            nc.sync.dma_start(outT01[64:128, half], oT1tmp[:, half])
            # head 2 alone
            pv2 = [
                pv_pool.tile([65, 512], F32, tag="pv", name=f"pv2_{qh}_{s_}")
                for s_ in range(2)
            ]
            for kt in range(TOKT):
                score_pv(2, qh, kt, pv2, f"{qh}_{kt}_2")
            normalize(2, qh, pv2)
            # project this query-half's tokens (overlaps the next qh's attention)
            outproj(2 * qh)
            outproj(2 * qh + 1)

        osb_pool.release()
        pv_pool.release()
        sc_pool.release()
        rc_pool.release()
        u_pool.release()

    nc.compile()
    return nc


def _get_nc():
    global _CACHED_NC
    if _CACHED_NC is None:
        _CACHED_NC = _build_nc()
    return _CACHED_NC


def _fold_cape(W, P):
    """W @ blockdiag(P) for 4x4 P repeated along channels: exact CAPE fold."""
    d = W.shape[1]
    W4 = W.reshape(W.shape[0], d // 4, 4)
    return np.einsum("cik,kj->cij", W4, P, optimize=True).reshape(W.shape[0], d)


def _prep_in_maps(hidden_states, p_out, p_out_inv, Wq, Wk, Wv, Wo):
    scale = HD ** -0.5
    hs2 = np.ascontiguousarray(hidden_states.reshape(S, D), dtype=np.float32)

    FEAT = PAD_HEADS * HD  # 1536
    Wq_eff = np.zeros((2, D, FEAT), np.float32)
    Wk_eff = np.zeros((2, D, FEAT), np.float32)
    for t in range(2):
        Wq_eff[t, :, :D] = _fold_cape(Wq, p_out_inv[0, t]) * scale
        Wk_eff[t, :, :D] = _fold_cape(Wk, p_out[0, t])
    Wv_pad = np.zeros((D, FEAT), np.float32)
    Wv_pad[:, :D] = Wv
    Wo_pad = np.zeros((FEAT, D), np.float32)
    Wo_pad[:D, :] = Wo

    def klayout(W, cols):
        # [1280, cols] -> [128, KT*cols] with ktile-major free dim
        return np.ascontiguousarray(
            W.reshape(KT, 128, cols).transpose(1, 0, 2).reshape(128, KT * cols)
        )

    ident = np.eye(128, dtype=np.float32)
    ones = np.ones((128, 64), np.float32)
    in_maps = []
    for c in range(N_CORES):
        A = c * HPC * HD
        blocks = []
        for t in range(2):
            blocks.append(klayout(Wq_eff[t][:, A : A + 128], 128))
            blocks.append(klayout(Wk_eff[t][:, A : A + 128], 128))
            blocks.append(
                klayout(
                    np.concatenate(
                        [
                            Wq_eff[t][:, A + 128 : A + 192],
                            Wk_eff[t][:, A + 128 : A + 192],
                        ],
                        axis=1,
                    ),
                    128,
                )
            )
        wv_l = klayout(
            np.concatenate(
                [Wv_pad[:, A : A + 192], np.zeros((D, 64), np.float32)], axis=1
            ),
            256,
        )
        wpack = np.ascontiguousarray(
            np.concatenate(blocks + [wv_l, ident, ones], axis=1)
        )
        assert wpack.shape == (128, WPACK_W)
        wopack = np.ascontiguousarray(
            np.concatenate(
                [
                    Wo_pad[A : A + 128, :],
                    np.concatenate(
                        [
                            Wo_pad[A + 128 : A + 192, :],
                            np.zeros((64, D), np.float32),
                        ],
                        axis=0,
                    ),
                ],
                axis=1,
            )
        )
        in_maps.append({"hs": hs2, "wpack": wpack, "wopack": wopack})
    return in_maps


def kernel(hidden_states, p_out, p_out_inv, Wq, Wk, Wv, Wo, bo):
    hidden_states = np.asarray(hidden_states, dtype=np.float32)
    in_maps = _prep_in_maps(
        hidden_states,
        np.asarray(p_out, np.float32),
        np.asarray(p_out_inv, np.float32),
        np.asarray(Wq, np.float32),
        np.asarray(Wk, np.float32),
        np.asarray(Wv, np.float32),
        np.asarray(Wo, np.float32),
    )
    nc = _get_nc()
    res = run_bass_kernel_spmd(nc, in_maps, core_ids=list(range(N_CORES)))
    acc = np.zeros((S, D), np.float32)
    for c in range(N_CORES):
        acc += res.results[c]["out"]
    acc += np.asarray(bo, np.float32)[None, :]
    out = acc.reshape(2, L, D) + hidden_states
    return out



# revision 6
# speedup vs baseline: 1.3414x; 1.3414x over previous
"""TRN2 Bass kernel for nn_AttentionStoreProcessor (dense transformer attention).

Full (unsharded) inputs in, full output out. Internally:
  - CAPE rotation + softmax scale folded into Wq/Wk on host; heads padded
    20 -> 24, tensor-parallel 3 heads/core across 8 cores, partial outputs
    summed on host (pad heads contribute exactly zero).
  - All operands bf16 on device (rel-err budget 2e-2).
  - hs transposed DRAM->SBUF by the DMA XBAR engine (no PE transposes).
  - Scores [ktok, q] on PE; exp on ACT (the critical engine, kept exp-only);
    PV computed transposed ([q, hd] out) with u as the stationary operand so
    the moving dim is hd+1 (65, with a fused ones-column for the softmax
    denominator) instead of q - roughly half the PE cost of attention.
  - Per-query softmax normalization is a per-partition scalar multiply.
  - Attention output re-transposed via one XBAR DMA per q-chunk; output
    projection from the transposed layout; residual/bias/core-sum on host.
  - q processed in chunks (896, 768, 384), each in two passes (heads 0+1,
    then head 2): outproj of chunk i overlaps the ACT-bound attention of
    chunk i+1; projection chunks 2/3 and v-projection tiles are woven into
    the first chunk's attention as PE filler granules.
"""
import numpy as np
from contextlib import ExitStack

import ml_dtypes
import concourse.bacc as bacc
import concourse.mybir as mybir
import concourse.tile as tile
from concourse.bass_utils import run_bass_kernel_spmd

F32 = mybir.dt.float32
BF16 = mybir.dt.bfloat16
AF = mybir.ActivationFunctionType

HEADS = 20
PAD_HEADS = 24
HPC = 3  # heads per core
N_CORES = 8
S = 2048
D = 1280
HD = 64
L = 1024
KT = D // 128  # 10 contraction tiles
TOKT = S // 128  # 16 token tiles

# q chunks: (qoff, n_qtiles); 7*65=455 fits one PSUM bank per head
QCHUNKS = [(0, 7), (896, 6), (1664, 3)]

_CACHED_NC = None


def _build_nc():
    nc = bacc.Bacc("TRN2", debug=False, num_devices=N_CORES)

    hs = nc.dram_tensor("hs", [S, D], BF16, kind="ExternalInput").ap()
    wg = nc.dram_tensor("wg", [128, 6 * KT * 128], BF16, kind="ExternalInput").ap()
    wv = nc.dram_tensor("wv", [128, KT * 192], BF16, kind="ExternalInput").ap()
    wo = nc.dram_tensor("wo", [128, 2560], BF16, kind="ExternalInput").ap()
    io = nc.dram_tensor("io", [128, 8], BF16, kind="ExternalInput").ap()
    out = nc.dram_tensor("out", [S, D], BF16, kind="ExternalOutput").ap()

    out_r = out.rearrange("(n p) d -> n p d", p=128)

    with (
        tile.TileContext(nc) as tc,
        ExitStack() as ctx,
        nc.allow_low_precision(reason="bf16 kernel; tolerance 2e-2"),
    ):
        persist = ctx.enter_context(tc.tile_pool(name="persist", bufs=1))

        # ---- small const + exp table warmup ----
        ones_sb = persist.tile([128, 8], BF16, tag="ones")
        nc.sync.dma_start(ones_sb[:], io[:])
        warm = persist.tile([128, 8], BF16, tag="warm")
        nc.scalar.activation(warm[:], ones_sb[:], AF.Exp)

        # ---- loads (SP/HWDGE), ordered by first use ----
        wg_sb = persist.tile([128, 6 * KT * 128], BF16, tag="wg")
        nc.sync.dma_start(wg_sb[:, 0 : 3 * 1280], wg[:, 0 : 3 * 1280])
        hsT = [
            persist.tile([128, S], BF16, tag=f"hsT{k}", name=f"hsT{k}")
            for k in range(KT)
        ]
        wv_sb = persist.tile([128, KT * 192], BF16, tag="wv")
        wo_sb = persist.tile([128, 2560], BF16, tag="wo")
        for half in range(2):
            for k in range(KT):
                nc.sync.dma_start_transpose(
                    hsT[k][:, half * 1024 : (half + 1) * 1024],
                    hs[half * 1024 : (half + 1) * 1024, k * 128 : (k + 1) * 128],
                )
            if half == 0:
                nc.sync.dma_start(wv_sb[:], wv[:])
                nc.sync.dma_start(
                    wg_sb[:, 3 * 1280 : 6 * 1280], wg[:, 3 * 1280 : 6 * 1280]
                )
        nc.sync.dma_start(wo_sb[:], wo[:])

        # ---- persistent SBUF state ----
        QA = persist.tile([128, S], BF16, tag="QA")  # rows 0:64 q_h0, 64:128 q_h1
        KA = persist.tile([128, S], BF16, tag="KA")  # rows 0:64 k_h0, 64:128 k_h1
        QK2 = persist.tile([128, S], BF16, tag="QK2")  # rows 0:64 q2, 64:128 k2
        QB2 = persist.tile([128, S], BF16, tag="QB2")  # rows 64:128 <- q2 shifted
        v_sb = persist.tile([128, TOKT, 195], BF16, tag="v_sb")
        oT = persist.tile([128, 2 * TOKT, 128], BF16, tag="oT")

        # ones columns of v (col 65h+64) for the softmax denominator
        v4 = v_sb.rearrange("p t (h c) -> p t h c", h=3)
        nc.gpsimd.memset(v4[:, :, :, 64:65], 1.0)

        u_pool = tc.alloc_tile_pool(name="u", bufs=6)
        attn_pool = tc.alloc_tile_pool(name="attn", bufs=2)
        rc_pool = tc.alloc_tile_pool(name="rc", bufs=4)
        osb_pool = tc.alloc_tile_pool(name="osb", bufs=3)

        # psum: tag "sc" (3 x 2 banks) shared by scores/proj/vproj/outproj;
        # tags pvA/pvB (1 bank each) hold per-(pass,head) PV accumulators
        ps = tc.alloc_tile_pool(name="ps", bufs=1, space="PSUM")

        def sc_tile(name, w=1024):
            return ps.tile([128, w], F32, tag="sc", bufs=3, name=name)

        def proj_group(ch, g):
            t = ch // 2
            qs = slice(ch * 512, (ch + 1) * 512)
            dest = (QA, KA, QK2)[g]
            pp = sc_tile(f"pp{ch}_{g}", 512)
            base = (t * 3 + g) * 1280
            for k in range(KT):
                nc.tensor.matmul(
                    pp[:, 0:512],
                    wg_sb[:, base + k * 128 : base + (k + 1) * 128],
                    hsT[k][:, qs],
                    start=(k == 0),
                    stop=(k == KT - 1),
                )
            nc.vector.tensor_copy(dest[:, qs], pp[:, 0:512])

        def vproj(n):
            vp = sc_tile(f"vp{n}", 192)
            for k in range(KT):
                nc.tensor.matmul(
                    vp[:, 0:192],
                    hsT[k][:, n * 128 : (n + 1) * 128],
                    wv_sb[:, k * 192 : (k + 1) * 192],
                    start=(k == 0),
                    stop=(k == KT - 1),
                )
            nc.vector.tensor_copy(
                v4[:, n, :, 0:64], vp[:, 0:192].rearrange("p (h c) -> p h c", h=3)
            )

        def head_ops(h):
            if h == 0:
                return KA, slice(0, 64), QA, slice(0, 64)
            if h == 1:
                return KA, slice(64, 128), QA, slice(64, 128)
            return QK2, slice(64, 128), QB2, slice(64, 128)

        def score_exp(qoff, cw, kt, h, name):
            ksrc, krows, qsrc, qrows = head_ops(h)
            sc = sc_tile(f"sc{name}")
            off = 0
            while off < cw:
                w = min(512, cw - off)
                nc.tensor.matmul(
                    sc[:, off : off + w],
                    ksrc[krows, kt * 128 : (kt + 1) * 128],
                    qsrc[qrows, qoff + off : qoff + off + w],
                    start=True,
                    stop=True,
                )
                off += w
            u = u_pool.tile([128, 1024], BF16, tag="u", name=f"u{name}")
            nc.scalar.activation(u[:, 0:cw], sc[:, 0:cw], AF.Exp)
            return u

        def pvt(kt, h, qts, u, pv_h):
            for qt in range(qts):
                nc.tensor.matmul(
                    pv_h[:, qt * 65 : (qt + 1) * 65],
                    u[:, qt * 128 : (qt + 1) * 128],
                    v_sb[:, kt, h * 65 : h * 65 + 65],
                    start=(kt == 0),
                    stop=(kt == TOKT - 1),
                )

        def normalize(h, qts, pv_h, attn):
            rc = rc_pool.tile([128, 8], F32, tag="rc", name=f"rc_{h}_{id(pv_h)}")
            pvv = pv_h[:, 0 : qts * 65].rearrange("p (q c) -> p q c", q=qts)
            nc.vector.reciprocal(rc[:, 0:qts], pvv[:, :, 64:65])
            for qt in range(qts):
                nc.vector.tensor_scalar_mul(
                    attn[:, qt, h * 64 : (h + 1) * 64],
                    pv_h[:, qt * 65 : qt * 65 + 64],
                    rc[:, qt : qt + 1],
                )

        _osb = {}

        def outproj_part(n, part, name):
            # part 0: cols 0:1024 (one psum tile, two 512 groups); part 1: 1024:1280
            if n not in _osb:
                _osb[n] = osb_pool.tile([128, D], BF16, tag="osb", name=f"ob{n}")
            if part == 0:
                op = sc_tile(f"op{name}a", 1024)
                for half in range(2):
                    cs = slice(half * 512, (half + 1) * 512)
                    nc.tensor.matmul(
                        op[:, cs], oT[:, 2 * n, :], wo_sb[:, cs], start=True, stop=False
                    )
                    nc.tensor.matmul(
                        op[:, cs],
                        oT[0:64, 2 * n + 1, :],
                        wo_sb[0:64, 1280 + half * 512 : 1280 + (half + 1) * 512],
                        start=False,
                        stop=True,
                    )
                nc.vector.tensor_copy(_osb[n][:, 0:1024], op[:, 0:1024])
            else:
                op = sc_tile(f"op{name}b", 256)
                nc.tensor.matmul(
                    op[:, 0:256], oT[:, 2 * n, :], wo_sb[:, 1024:1280],
                    start=True, stop=False,
                )
                nc.tensor.matmul(
                    op[:, 0:256],
                    oT[0:64, 2 * n + 1, :],
                    wo_sb[0:64, 2304:2560],
                    start=False,
                    stop=True,
                )
                nc.vector.tensor_copy(_osb[n][:, 1024:1280], op[:, 0:256])
                nc.sync.dma_start(out_r[n], _osb[n][:])
                del _osb[n]

        # ---- emission ----
        # head: projections for chunks 0,1 + first three v tiles
        for ch in (0, 1):
            for g in range(3):
                proj_group(ch, g)
        for n in (0, 1, 2):
            vproj(n)

        # granules woven into chunk-0 attention: remaining vproj tiles (vp_n
        # due before its kt step), proj chunks 2,3 (due before kt 8 / 12),
        # and the q2 partition shift (due before pass 2)
        granules = [
            [("v", 3)],
            [("v", 4), ("p", 2, 0)],
            [("v", 5), ("p", 2, 1)],
            [("v", 6), ("p", 2, 2)],
            [("v", 7)],
            [("v", 8), ("p", 3, 0)],
            [("v", 9), ("p", 3, 1)],
            [("v", 10), ("p", 3, 2)],
            [("v", 11), ("q2",)],
            [("v", 12)],
            [("v", 13)],
            [("v", 14)],
            [("v", 15)],
        ]

        def run_granule(g):
            if g[0] == "v":
                vproj(g[1])
            elif g[0] == "p":
                proj_group(g[1], g[2])
            else:
                nc.sync.dma_start(QB2[64:128, :], QK2[0:64, :])

        def attention(ci, fillers):
            """fillers: list of thunks consumed as PE filler, one per kt step."""
            qoff, qts = QCHUNKS[ci]
            cw = qts * 128
            attn = attn_pool.tile([128, qts, 256], BF16, tag="attn", name=f"at{ci}")
            nc.gpsimd.memset(attn[:, :, 192:256], 0)
            fi = 0
            for pi, heads in enumerate(((0, 1), (2,))):
                pvs = {
                    h: ps.tile(
                        [128, 512], F32, tag="pvA" if j == 0 else "pvB",
                        name=f"pv{ci}_{h}",
                    )
                    for j, h in enumerate(heads)
                }
                us = {}
                for kt in range(TOKT):
                    for h in heads:
                        us[(kt, h)] = score_exp(qoff, cw, kt, h, f"{ci}_{kt}_{h}")
                        if kt > 0:
                            pvt(kt - 1, h, qts, us.pop((kt - 1, h)), pvs[h])
                    if fi < len(fillers):
                        for g in fillers[fi]:
                            run_granule(g)
                        fi += 1
                for h in heads:
                    pvt(TOKT - 1, h, qts, us.pop((TOKT - 1, h)), pvs[h])
                    normalize(h, qts, pvs[h], attn)
            while fi < len(fillers):
                for g in fillers[fi]:
                    run_granule(g)
                fi += 1
            qtg = qoff // 128
            nc.sync.dma_start_transpose(
                oT[:, 2 * qtg : 2 * (qtg + qts), :], attn[:, :, :]
            )
            return [qtg + i for i in range(qts)]

        tiles0 = attention(0, granules)
        op0 = [[("o", n, p)] for n in tiles0 for p in (0, 1)]

        def run_g2(g):
            if g[0] == "o":
                outproj_part(g[1], g[2], f"f{g[1]}")
            else:
                run_granule(g)

        def attention2(ci, fillers, fill_start):
            qoff, qts = QCHUNKS[ci]
            cw = qts * 128
            attn = attn_pool.tile([128, qts, 256], BF16, tag="attn", name=f"at{ci}")
            nc.gpsimd.memset(attn[:, :, 192:256], 0)
            fi = 0
            step = 0
            for pi, heads in enumerate(((0, 1), (2,))):
                pvs = {
                    h: ps.tile(
                        [128, 512], F32, tag="pvA" if j == 0 else "pvB",
                        name=f"pv{ci}_{h}",
                    )
                    for j, h in enumerate(heads)
                }
                us = {}
                for kt in range(TOKT):
                    for h in heads:
                        us[(kt, h)] = score_exp(qoff, cw, kt, h, f"{ci}_{kt}_{h}")
                        if kt > 0:
                            pvt(kt - 1, h, qts, us.pop((kt - 1, h)), pvs[h])
                    if step >= fill_start and fi < len(fillers):
                        for g in fillers[fi]:
                            run_g2(g)
                        fi += 1
                    step += 1
                for h in heads:
                    pvt(TOKT - 1, h, qts, us.pop((TOKT - 1, h)), pvs[h])
                    normalize(h, qts, pvs[h], attn)
            while fi < len(fillers):
                for g in fillers[fi]:
                    run_g2(g)
                fi += 1
            qtg = qoff // 128
            nc.sync.dma_start_transpose(
                oT[:, 2 * qtg : 2 * (qtg + qts), :], attn[:, :, :]
            )
            return [qtg + i for i in range(qts)]

        tiles1 = attention2(1, op0, 2)
        op1 = [[("o", n, p)] for n in tiles1 for p in (0, 1)]
        tiles2 = attention2(2, op1, 2)
        for n in tiles2:
            outproj_part(n, 0, f"t{n}")
            outproj_part(n, 1, f"t{n}")

        osb_pool.release()
        rc_pool.release()
        attn_pool.release()
        u_pool.release()
        ps.release()

    nc.compile()
    return nc


def _get_nc():
    global _CACHED_NC
    if _CACHED_NC is None:
        _CACHED_NC = _build_nc()
    return _CACHED_NC


def _fold_cape(W, P):
    """W @ blockdiag(P) for 4x4 P repeated along channels: exact CAPE fold."""
    d = W.shape[1]
    W4 = W.reshape(W.shape[0], d // 4, 4)
    return np.einsum("cik,kj->cij", W4, P, optimize=True).reshape(W.shape[0], d)


def _klayout(W, cols):
    # [1280, cols] -> [128, KT*cols] with ktile-major free dim
    return np.ascontiguousarray(
        W.reshape(KT, 128, cols).transpose(1, 0, 2).reshape(128, KT * cols)
    )


def _bf16(x):
    return np.ascontiguousarray(x.astype(ml_dtypes.bfloat16))


def _prep_in_maps(hidden_states, p_out, p_out_inv, Wq, Wk, Wv, Wo):
    scale = HD ** -0.5
    hs2 = np.ascontiguousarray(hidden_states.reshape(S, D), dtype=np.float32)

    FEAT = PAD_HEADS * HD  # 1536
    Wq_eff = np.zeros((2, D, FEAT), np.float32)
    Wk_eff = np.zeros((2, D, FEAT), np.float32)
    for t in range(2):
        Wq_eff[t, :, :D] = _fold_cape(Wq, p_out_inv[0, t]) * scale
        Wk_eff[t, :, :D] = _fold_cape(Wk, p_out[0, t])
    Wv_pad = np.zeros((D, FEAT), np.float32)
    Wv_pad[:, :D] = Wv
    Wo_pad = np.zeros((FEAT, D), np.float32)
    Wo_pad[:D, :] = Wo

    hs_bf = _bf16(hs2)
    io = np.ones((128, 8), np.float32)
    in_maps = []
    for c in range(N_CORES):
        A = c * HPC * HD
        blocks = []
        for t in range(2):
            blocks.append(_klayout(Wq_eff[t][:, A : A + 128], 128))
            blocks.append(_klayout(Wk_eff[t][:, A : A + 128], 128))
            blocks.append(
                _klayout(
                    np.concatenate(
                        [
                            Wq_eff[t][:, A + 128 : A + 192],
                            Wk_eff[t][:, A + 128 : A + 192],
                        ],
                        axis=1,
                    ),
                    128,
                )
            )
        wgl = np.concatenate(blocks, axis=1)
        wvl = _klayout(Wv_pad[:, A : A + 192], 192)
        wol = np.concatenate(
            [
                Wo_pad[A : A + 128, :],
                np.concatenate(
                    [Wo_pad[A + 128 : A + 192, :], np.zeros((64, D), np.float32)],
                    axis=0,
                ),
            ],
            axis=1,
        )
        in_maps.append(
            {
                "hs": hs_bf,
                "wg": _bf16(wgl),
                "wv": _bf16(wvl),
                "wo": _bf16(wol),
                "io": _bf16(io),
            }
        )
    return in_maps


def kernel(hidden_states, p_out, p_out_inv, Wq, Wk, Wv, Wo, bo):
    hidden_states = np.asarray(hidden_states, dtype=np.float32)
    in_maps = _prep_in_maps(
        hidden_states,
        np.asarray(p_out, np.float32),
        np.asarray(p_out_inv, np.float32),
        np.asarray(Wq, np.float32),
        np.asarray(Wk, np.float32),
        np.asarray(Wv, np.float32),
        np.asarray(Wo, np.float32),
    )
    nc = _get_nc()
    res = run_bass_kernel_spmd(nc, in_maps, core_ids=list(range(N_CORES)))
    acc = np.zeros((S, D), np.float32)
    for c in range(N_CORES):
        acc += np.asarray(res.results[c]["out"], dtype=np.float32)
    acc += np.asarray(bo, np.float32)[None, :]
    out = acc.reshape(2, L, D) + hidden_states
    return out
